# revision 6
# baseline (speedup 1.0000x reference)
"""Multi-head attention kernel for 8 Trainium2 NeuronCores (v2, all-bf16 PE).

Problem: x[4, 2048, 1024], 16 heads x 64 head-dim MHA (QKV proj -> softmax
attention -> out proj), fp32 in/out.

Sharding: 8 cores = 4 batches x 2 head-groups. Core c handles batch c//2 and
heads (c%2)*8 .. (c%2)*8+7. Each core computes a partial output [2048, 1024]
(its 8 heads through Wo); the host sums the two partials per batch + bo.

vs the fp32r baseline (measured on this hardware):
  - Host pre-transposes x to x^T bf16 and casts all weights to bf16: no PE
    transposes, no staging casts, half the DMA bytes.
  - All matmuls use bf16 (or fp16) operands: ~133-184 ns per
    [128,128]x[128,512] vs 285+ ns for fp32r.
  - K^T is stored zero-padded per head (ZKA rows 0:64 = K_A, rows 64:128 = 0;
    ZKB opposite) so score matmuls use full-height 128-row stationaries:
    half-height (K=64) stationaries measured 2.7x slower (440 ns).
  - exp -> fp16 on ScalarE (688 ns per [128,1024]; bf16 out was 958, fp32
    958+DVE cast); Vaug is fp16 to match the attnV operand dtype.
  - Three dense phases: QKV projections, then attention units (s2 psum
    rotation depth 3, attnV lagging the exp frontier by 2 kt), then output
    projection. Interleaving projection work into the attention stream was
    measured strictly worse (each kt step pays a ~300-450 ns serialization
    tax on top of max(PE, ACT) work).
  - Input DMAs merged into a few big strided transfers (HWDGE setup is
    ~625 ns each); one-time memsets run on the idle Pool engine.
"""

import numpy as np

B = 4
SEQ = 2048
DIM = 1024
NH_LOC = 8      # heads per core
HID = 64
HDL = NH_LOC * HID  # 512
N_CORES = 8

_PROG = None
PHASES = "all"   # "qkv" | "attn" | "all" — debugging aid for phase timing


def _build_program(seq=SEQ, reps=1):
    import contextlib

    import concourse.bass as bass
    import concourse.mybir as mybir
    import concourse.tile as tile
    from concourse import bacc

    FP32 = mybir.dt.float32
    BF16 = mybir.dt.bfloat16

    seq_t = seq // 128            # 16
    dim_t = DIM // 128            # 8
    n_qc = seq // 512             # 4
    n_hp = NH_LOC // 2            # 4
    n_m = HDL // 128              # 4

    nc = bacc.Bacc()
    xt_d = nc.declare_dram_parameter("xt", [DIM, seq], BF16, isOutput=False)
    wq_d = nc.declare_dram_parameter("wq", [DIM, HDL], BF16, isOutput=False)
    wk_d = nc.declare_dram_parameter("wk", [DIM, HDL], BF16, isOutput=False)
    wv_d = nc.declare_dram_parameter("wv", [DIM, HDL], BF16, isOutput=False)
    bq_d = nc.declare_dram_parameter("bq", [HDL], FP32, isOutput=False)
    bk_d = nc.declare_dram_parameter("bk", [HDL], FP32, isOutput=False)
    bv_d = nc.declare_dram_parameter("bv", [HDL], FP32, isOutput=False)
    wo_d = nc.declare_dram_parameter("wo", [HDL, DIM], BF16, isOutput=False)
    out_d = nc.declare_dram_parameter("out", [seq, DIM], FP32, isOutput=True)
    rrs_d = nc.dram_tensor("rrs", [n_hp, n_qc, 2, 512], FP32)

    with tile.TileContext(nc, pool_alloc_mode="queue") as tc:
        with tc.tile_pool(name="persist", bufs=1) as persist:
            xTall = persist.tile([128, dim_t * seq], BF16, tag="xtall",
                                 name="xtall")
            xT = [xTall[:, d*seq:(d+1)*seq] for d in range(dim_t)]
            QT = [persist.tile([128, seq], BF16, tag=f"qt{m}", name=f"qt{m}")
                  for m in range(n_m)]
            ZKA = [[persist.tile([128, 128], BF16, tag=f"zka{m}_{t}",
                                 name=f"zka{m}_{t}") for t in range(seq_t)]
                   for m in range(n_m)]
            ZKB = [[persist.tile([128, 128], BF16, tag=f"zkb{m}_{t}",
                                 name=f"zkb{m}_{t}") for t in range(seq_t)]
                   for m in range(n_m)]
            FP16 = mybir.dt.float16
            Vaug = [persist.tile([128, NH_LOC * (HID + 1)], FP16,
                                 tag=f"va{st}", name=f"va{st}")
                    for st in range(seq_t)]
            UT = [[persist.tile([128, 512], BF16, tag=f"ut{hp}_{q}",
                                name=f"ut{hp}_{q}")
                   for q in range(n_qc)] for hp in range(n_hp)]
            # weights as separate small tiles: big-tile stationary slices
            # measured ~2x slower weight loads than small dedicated tiles
            wqr = [persist.tile([128, HDL], BF16, tag=f"wq{d}",
                                name=f"wq{d}") for d in range(dim_t)]
            wkr = [persist.tile([128, HDL], BF16, tag=f"wk{d}",
                                name=f"wk{d}") for d in range(dim_t)]
            wvr = [persist.tile([128, HDL], BF16, tag=f"wv{d}",
                                name=f"wv{d}") for d in range(dim_t)]
            woall = persist.tile([128, n_hp * DIM], BF16, tag="woa",
                                 name="woa")
            wor = [woall[:, hp*DIM:(hp+1)*DIM] for hp in range(n_hp)]
            bq_sb = persist.tile([128, n_m], FP32)
            bk_sb = persist.tile([128, n_m], FP32)

            # one-time setup (Pool engine — keeps DVE free for the body):
            # zero halves of ZK, ones column of Vaug, biases
            for m in range(n_m):
                for t in range(seq_t):
                    nc.gpsimd.memset(ZKA[m][t][64:128, :], 0.0)
                    nc.gpsimd.memset(ZKB[m][t][0:64, :], 0.0)
            for st in range(seq_t):
                va3 = Vaug[st][:].rearrange("p (h c) -> p h c", c=HID + 1)
                nc.gpsimd.memset(va3[:, :, HID:HID + 1], 1.0)
            nc.sync.dma_start(out=bq_sb[:],
                              in_=bq_d[:].rearrange("(m p) -> p m", p=128))
            nc.sync.dma_start(out=bk_sb[:],
                              in_=bk_d[:].rearrange("(m p) -> p m", p=128))
            bv_bc = persist.tile([128, HDL], FP32)
            bv_ap = bv_d[:]
            nc.sync.dma_start(
                out=bv_bc[:],
                in_=bass.AP(tensor=bv_ap.tensor, offset=bv_ap.offset,
                            ap=[[0, 128], [1, HDL]]),
            )

            rep_ctx = tc.For_i(0, reps, 1) if reps > 1 else contextlib.nullcontext()
            with rep_ctx:
                _build_body(nc, tc, bass, mybir, locals())

    nc.compile()
    return nc


def _build_body(nc, tc, bass, mybir, env):
    FP32 = mybir.dt.float32
    BF16 = mybir.dt.bfloat16
    FP16 = mybir.dt.float16
    Exp = mybir.ActivationFunctionType.Exp
    Alu = mybir.AluOpType
    seq = env["seq"]
    seq_t, dim_t = env["seq_t"], env["dim_t"]
    n_qc, n_hp, n_m = env["n_qc"], env["n_hp"], env["n_m"]
    xT, QT, ZKA, ZKB = env["xT"], env["QT"], env["ZKA"], env["ZKB"]
    Vaug, UT = env["Vaug"], env["UT"]
    wqr, wkr, wvr, wor = env["wqr"], env["wkr"], env["wvr"], env["wor"]
    xTall, woall = env["xTall"], env["woall"]
    bq_sb, bk_sb, bv_bc = env["bq_sb"], env["bk_sb"], env["bv_bc"]
    xt_d, wq_d, wk_d, wv_d = env["xt_d"], env["wq_d"], env["wk_d"], env["wv_d"]
    wo_d, out_d, rrs_d = env["wo_d"], env["out_d"], env["rrs_d"]

    with (
        tc.tile_pool(name="epool", bufs=6) as epool,
        tc.tile_pool(name="rbpool", bufs=4) as rbpool,
        tc.tile_pool(name="rpool", bufs=4) as rpool,
        tc.tile_pool(name="outstage", bufs=3) as outstage,
    ):
        # ---- input DMAs: few big strided transfers (HWDGE setup is ~625ns
        # per dma_start, so merge aggressively). xT lands in column chunks so
        # the first projections unblock after ~1MB.
        nc.sync.dma_start(
            out=wqall[:].rearrange("p (d c) -> p d c", c=HDL),
            in_=wq_d[:].rearrange("(d p) c -> p d c", p=128))
        xT3 = xTall[:].rearrange("p (d c) -> p d c", c=seq)
        nc.sync.dma_start(
            out=xT3[:, :, 0:512],
            in_=xt_d[:, 0:512].rearrange("(d p) c -> p d c", p=128))
        nc.sync.dma_start(
            out=wkall[:].rearrange("p (d c) -> p d c", c=HDL),
            in_=wk_d[:].rearrange("(d p) c -> p d c", p=128))
        nc.sync.dma_start(
            out=wvall[:].rearrange("p (d c) -> p d c", c=HDL),
            in_=wv_d[:].rearrange("(d p) c -> p d c", p=128))
        for cc in range(1, n_qc):
            cs = slice(cc * 512, (cc + 1) * 512)
            nc.sync.dma_start(
                out=xT3[:, :, cs],
                in_=xt_d[:, cs].rearrange("(d p) c -> p d c", p=128))
        nc.sync.dma_start(
            out=woall[:].rearrange("p (h c) -> p h c", c=DIM),
            in_=wo_d[:].rearrange("(h p) c -> p h c", p=128))

        # ---- phase 1: dense QKV projections (ScalarE idle; PE runs b2b) ----
        with tc.tile_pool(name="qkvp", bufs=1, space="PSUM") as qkvp:
            chain_ctr = [0]

            def bgtile():
                t = qkvp.tile([128, 512], FP32,
                              tag=f"bg{chain_ctr[0] % 8}",
                              name=f"bg{chain_ctr[0] % 8}")
                chain_ctr[0] += 1
                return t

            def emit_q(m, qc):
                qp = bgtile()
                for d in range(dim_t):
                    nc.tensor.matmul(qp[:], wqr[d][:, m*128:(m+1)*128],
                                     xT[d][:, qc*512:(qc+1)*512],
                                     start=(d == 0), stop=(d == dim_t - 1))
                nc.vector.tensor_scalar(QT[m][:, qc*512:(qc+1)*512], qp[:],
                                        bq_sb[:, m:m+1], None, Alu.add)

            def emit_k(m, kc):
                kp = bgtile()
                for d in range(dim_t):
                    nc.tensor.matmul(kp[:], wkr[d][:, m*128:(m+1)*128],
                                     xT[d][:, kc*512:(kc+1)*512],
                                     start=(d == 0), stop=(d == dim_t - 1))
                # (K + bias) * 1/sqrt(HID), split per head into per-kt
                # zero-padded small tiles (big-tile slice stationaries load
                # slower on this HW)
                for j in range(4):
                    t = kc * 4 + j
                    js = slice(j * 128, (j + 1) * 128)
                    nc.vector.tensor_scalar(ZKA[m][t][0:64, :],
                                            kp[0:64, js],
                                            bk_sb[0:64, m:m+1], 0.125,
                                            Alu.add, Alu.mult)
                    nc.vector.tensor_scalar(ZKB[m][t][64:128, :],
                                            kp[64:128, js],
                                            bk_sb[64:128, m:m+1], 0.125,
                                            Alu.add, Alu.mult)

            def emit_v(st):
                vp = bgtile()
                for d in range(dim_t):
                    nc.tensor.matmul(vp[:], xT[d][:, st*128:(st+1)*128],
                                     wvr[d][:],
                                     start=(d == 0), stop=(d == dim_t - 1))
                va3 = Vaug[st][:].rearrange("p (h c) -> p h c", c=HID + 1)
                nc.vector.tensor_tensor(
                    va3[:, :, 0:HID],
                    vp[:].rearrange("p (h c) -> p h c", c=HID),
                    bv_bc[:].rearrange("p (h c) -> p h c", c=HID),
                    Alu.add)

            # PE warm-up during the input DMAs: garbage matmuls keep the PE
            # busy (and the p-state ramping) until the first weights land.
            for i in range(18):
                wp = bgtile()
                nc.tensor.matmul(wp[:, 0:128], ZKA[0][0][64:128, :],
                                 ZKA[0][1][64:128, :],
                                 start=True, stop=True)

            # column chunk 0 lands first: emit its projections, then the rest
            for m in range(n_m):
                emit_q(m, 0)
                emit_k(m, 0)
            for st in range(4):
                emit_v(st)
            for c in range(1, n_qc):
                for m in range(n_m):
                    emit_q(m, c)
                    emit_k(m, c)
                for st in range(4 * c, 4 * c + 4):
                    emit_v(st)

        # ---- phase 2: attention units (no interleaved projection work; s2
        # rotation depth 3 so scores never wait on the exp 2 steps back) ----
        def finish(hp, qc, ua, ub):
            for hi, (ui, rowbase) in enumerate(((ua, 0), (ub, 64))):
                usb = rbpool.tile([HID + 1, 512], FP32, tag="usb", name="usb")
                nc.vector.tensor_copy(usb[:], ui[:])
                rr = rpool.tile([1, 512], FP32, tag="rr", name="rr")
                nc.vector.reciprocal(rr[:], usb[HID:HID+1, :])
                slot = rrs_d[hp, qc, hi, :]
                nc.sync.dma_start(out=slot, in_=rr[0:1, :])
                rb = rbpool.tile([HID, 512], FP32, tag="rb", name="rb")
                nc.sync.dma_start(
                    out=rb[:],
                    in_=bass.AP(tensor=slot.tensor, offset=slot.offset,
                                ap=[[0, HID], [1, 512]]))
                nc.vector.tensor_tensor(
                    UT[hp][qc][rowbase:rowbase+HID, :],
                    usb[0:HID, :], rb[:], Alu.mult)

        with (
            tc.tile_pool(name="sps", bufs=1, space="PSUM") as sps,
            tc.tile_pool(name="ups", bufs=1, space="PSUM") as ups,
        ):
            ktg = [0]   # global kt counter: s2 tag rotation continues
                        # across unit boundaries so the pipeline never drains

            def emit_unit(hp, qc):
                vca = 2 * hp * (HID + 1)
                vcb = (2 * hp + 1) * (HID + 1)
                ua = ups.tile([HID + 1, 512], FP32, tag="ua", name="ua")
                ub = ups.tile([HID + 1, 512], FP32, tag="ub", name="ub")
                qcols = slice(qc * 512, (qc + 1) * 512)
                # attnV lags the score/exp frontier by 2 kt steps so the PE
                # is never queued right behind an exp it would wait on.
                lag = []
                for kt in range(seq_t):
                    s2 = sps.tile([128, 1024], FP32, tag=f"s{ktg[0] % 3}",
                                  name=f"s{ktg[0] % 3}")
                    ktg[0] += 1
                    nc.tensor.matmul(s2[:, 0:512], ZKA[hp][kt][:],
                                     QT[hp][:, qcols], start=True, stop=True)
                    nc.tensor.matmul(s2[:, 512:1024], ZKB[hp][kt][:],
                                     QT[hp][:, qcols], start=True, stop=True)
                    e2 = epool.tile([128, 1024], FP16, tag="e2", name="e2")
                    nc.scalar.activation(e2[:], s2[:], Exp)
                    if len(lag) >= 2:
                        lag.pop(0)()

                    def attn(e2=e2, kt=kt, ua=ua, ub=ub, vca=vca, vcb=vcb):
                        nc.tensor.matmul(ua[:], Vaug[kt][:, vca:vca+HID+1],
                                         e2[:, 0:512],
                                         start=(kt == 0),
                                         stop=(kt == seq_t - 1))
                        nc.tensor.matmul(ub[:], Vaug[kt][:, vcb:vcb+HID+1],
                                         e2[:, 512:1024],
                                         start=(kt == 0),
                                         stop=(kt == seq_t - 1))
                    lag.append(attn)
                for fn in lag:
                    fn()
                finish(hp, qc, ua, ub)

            for u in range(16):
                hp, qc = divmod(u, 4)
                emit_unit(hp, qc)

        # ---- phase 3: dense output projection ----
        with tc.tile_pool(name="ops", bufs=1, space="PSUM") as ops:
            op_ctr = [0]
            for qc in range(n_qc):
                for sti in range(4):
                    st = qc * 4 + sti
                    ot = outstage.tile([128, DIM], FP32, tag="ot", name="ot")
                    for oc in range(2):
                        op = ops.tile([128, 512], FP32,
                                      tag=f"op{op_ctr[0] % 8}",
                                      name=f"op{op_ctr[0] % 8}")
                        op_ctr[0] += 1
                        for hp in range(n_hp):
                            nc.tensor.matmul(
                                op[:], UT[hp][qc][:, sti*128:(sti+1)*128],
                                wor[hp][:, oc*512:(oc+1)*512],
                                start=(hp == 0), stop=(hp == n_hp - 1))
                        nc.vector.tensor_copy(ot[:, oc*512:(oc+1)*512], op[:])
                    (nc.sync if st % 2 == 0 else nc.gpsimd).dma_start(
                        out=out_d[st*128:(st+1)*128, :], in_=ot[:])


def _get_program():
    global _PROG
    if _PROG is None:
        _PROG = _build_program()
    return _PROG


def _make_in_maps(inputs):
    import ml_dtypes
    bf16 = ml_dtypes.bfloat16
    x = np.asarray(inputs["x"], dtype=np.float32)
    xts = [np.ascontiguousarray(x[b].T).astype(bf16) for b in range(B)]
    wq = np.asarray(inputs["Wq"], np.float32).astype(bf16)
    wk = np.asarray(inputs["Wk"], np.float32).astype(bf16)
    wv = np.asarray(inputs["Wv"], np.float32).astype(bf16)
    wo = np.asarray(inputs["Wo"], np.float32).astype(bf16)
    in_maps = []
    for c in range(N_CORES):
        b, g = divmod(c, 2)
        sl = slice(g * HDL, (g + 1) * HDL)
        in_maps.append({
            "xt": xts[b],
            "wq": np.ascontiguousarray(wq[:, sl]),
            "wk": np.ascontiguousarray(wk[:, sl]),
            "wv": np.ascontiguousarray(wv[:, sl]),
            "bq": np.ascontiguousarray(np.asarray(inputs["bq"], np.float32)[sl]),
            "bk": np.ascontiguousarray(np.asarray(inputs["bk"], np.float32)[sl]),
            "bv": np.ascontiguousarray(np.asarray(inputs["bv"], np.float32)[sl]),
            "wo": np.ascontiguousarray(wo[sl, :]),
        })
    return in_maps


def kernel(x, Wq, bq, Wk, bk, Wv, bv, Wo, bo):
    from concourse.bass_utils import run_bass_kernel_spmd

    bo = np.asarray(bo, dtype=np.float32)
    nc = _get_program()
    in_maps = _make_in_maps(dict(x=x, Wq=Wq, bq=bq, Wk=Wk, bk=bk, Wv=Wv,
                                 bv=bv, Wo=Wo, bo=bo))
    res = run_bass_kernel_spmd(nc, in_maps, core_ids=list(range(N_CORES)))
    out = np.empty((B, SEQ, DIM), dtype=np.float32)
    for b in range(B):
        out[b] = res.results[2 * b]["out"] + res.results[2 * b + 1]["out"] + bo
    return out


# revision 7
# speedup vs baseline: 1.1242x; 1.1242x over previous
"""Multi-head attention kernel for 8 Trainium2 NeuronCores (v2, all-bf16 PE).

Problem: x[4, 2048, 1024], 16 heads x 64 head-dim MHA (QKV proj -> softmax
attention -> out proj), fp32 in/out.

Sharding: 8 cores = 4 batches x 2 head-groups. Core c handles batch c//2 and
heads (c%2)*8 .. (c%2)*8+7. Each core computes a partial output [2048, 1024]
(its 8 heads through Wo); the host sums the two partials per batch + bo.

vs the fp32r baseline (measured on this hardware):
  - Host pre-transposes x to x^T bf16 and casts all weights to bf16: no PE
    transposes, no staging casts, half the DMA bytes.
  - All matmuls use bf16 (or fp16) operands: ~133-184 ns per
    [128,128]x[128,512] vs 285+ ns for fp32r.
  - K^T is stored zero-padded per head (ZKA rows 0:64 = K_A, rows 64:128 = 0;
    ZKB opposite) so score matmuls use full-height 128-row stationaries:
    half-height (K=64) stationaries measured 2.7x slower (440 ns).
  - exp -> fp16 on ScalarE (688 ns per [128,1024]; bf16 out was 958, fp32
    958+DVE cast); Vaug is fp16 to match the attnV operand dtype.
  - Three dense phases: QKV projections, then attention units (s2 psum
    rotation depth 3, attnV lagging the exp frontier by 2 kt), then output
    projection. Interleaving projection work into the attention stream was
    measured strictly worse (each kt step pays a ~300-450 ns serialization
    tax on top of max(PE, ACT) work).
  - Input DMAs merged into a few big strided transfers (HWDGE setup is
    ~625 ns each); one-time memsets run on the idle Pool engine.
"""

import numpy as np

B = 4
SEQ = 2048
DIM = 1024
NH_LOC = 8      # heads per core
HID = 64
HDL = NH_LOC * HID  # 512
N_CORES = 8

_PROG = None
PHASES = "all"   # "qkv" | "attn" | "all" — debugging aid for phase timing


def _build_program(seq=SEQ, reps=1):
    import contextlib

    import concourse.bass as bass
    import concourse.mybir as mybir
    import concourse.tile as tile
    from concourse import bacc

    FP32 = mybir.dt.float32
    BF16 = mybir.dt.bfloat16

    seq_t = seq // 128            # 16
    dim_t = DIM // 128            # 8
    n_qc = seq // 512             # 4
    n_hp = NH_LOC // 2            # 4
    n_m = HDL // 128              # 4

    nc = bacc.Bacc()
    xt_d = nc.declare_dram_parameter("xt", [DIM, seq], BF16, isOutput=False)
    wq_d = nc.declare_dram_parameter("wq", [DIM, HDL], BF16, isOutput=False)
    wk_d = nc.declare_dram_parameter("wk", [DIM, HDL], BF16, isOutput=False)
    wv_d = nc.declare_dram_parameter("wv", [DIM, HDL], BF16, isOutput=False)
    bq_d = nc.declare_dram_parameter("bq", [HDL], FP32, isOutput=False)
    bk_d = nc.declare_dram_parameter("bk", [HDL], FP32, isOutput=False)
    bv_d = nc.declare_dram_parameter("bv", [HDL], FP32, isOutput=False)
    wo_d = nc.declare_dram_parameter("wo", [HDL, DIM], BF16, isOutput=False)
    out_d = nc.declare_dram_parameter("out", [seq, DIM], FP32, isOutput=True)
    rrs_d = nc.dram_tensor("rrs", [n_hp, n_qc, 2, 512], FP32)

    with tile.TileContext(nc, pool_alloc_mode="queue") as tc:
        with tc.tile_pool(name="persist", bufs=1) as persist:
            xTall = persist.tile([128, dim_t * seq], BF16, tag="xtall",
                                 name="xtall")
            xT = [xTall[:, d*seq:(d+1)*seq] for d in range(dim_t)]
            QT = [persist.tile([128, seq], BF16, tag=f"qt{m}", name=f"qt{m}")
                  for m in range(n_m)]
            ZKA = [[persist.tile([128, 128], BF16, tag=f"zka{m}_{t}",
                                 name=f"zka{m}_{t}") for t in range(seq_t)]
                   for m in range(n_m)]
            ZKB = [[persist.tile([128, 128], BF16, tag=f"zkb{m}_{t}",
                                 name=f"zkb{m}_{t}") for t in range(seq_t)]
                   for m in range(n_m)]
            FP16 = mybir.dt.float16
            Vaug = [persist.tile([128, NH_LOC * (HID + 1)], FP16,
                                 tag=f"va{st}", name=f"va{st}")
                    for st in range(seq_t)]
            UT = [[persist.tile([128, 512], BF16, tag=f"ut{hp}_{q}",
                                name=f"ut{hp}_{q}")
                   for q in range(n_qc)] for hp in range(n_hp)]
            # weights as separate small tiles: big-tile stationary slices
            # measured ~2x slower weight loads than small dedicated tiles
            wqr = [persist.tile([128, HDL], BF16, tag=f"wq{d}",
                                name=f"wq{d}") for d in range(dim_t)]
            wkr = [persist.tile([128, HDL], BF16, tag=f"wk{d}",
                                name=f"wk{d}") for d in range(dim_t)]
            wvr = [persist.tile([128, HDL], BF16, tag=f"wv{d}",
                                name=f"wv{d}") for d in range(dim_t)]
            woall = persist.tile([128, n_hp * DIM], BF16, tag="woa",
                                 name="woa")
            wor = [woall[:, hp*DIM:(hp+1)*DIM] for hp in range(n_hp)]
            bq_sb = persist.tile([128, n_m], FP32)
            bk_sb = persist.tile([128, n_m], FP32)

            # one-time setup (Pool engine — keeps DVE free for the body):
            # zero halves of ZK, ones column of Vaug, biases
            for m in range(n_m):
                for t in range(seq_t):
                    nc.gpsimd.memset(ZKA[m][t][64:128, :], 0.0)
                    nc.gpsimd.memset(ZKB[m][t][0:64, :], 0.0)
            for st in range(seq_t):
                va3 = Vaug[st][:].rearrange("p (h c) -> p h c", c=HID + 1)
                nc.gpsimd.memset(va3[:, :, HID:HID + 1], 1.0)
            nc.sync.dma_start(out=bq_sb[:],
                              in_=bq_d[:].rearrange("(m p) -> p m", p=128))
            nc.sync.dma_start(out=bk_sb[:],
                              in_=bk_d[:].rearrange("(m p) -> p m", p=128))
            bv_bc = persist.tile([128, HDL], FP32)
            bv_ap = bv_d[:]
            nc.sync.dma_start(
                out=bv_bc[:],
                in_=bass.AP(tensor=bv_ap.tensor, offset=bv_ap.offset,
                            ap=[[0, 128], [1, HDL]]),
            )

            rep_ctx = tc.For_i(0, reps, 1) if reps > 1 else contextlib.nullcontext()
            with rep_ctx:
                _build_body(nc, tc, bass, mybir, locals())

    nc.compile()
    return nc


def _build_body(nc, tc, bass, mybir, env):
    FP32 = mybir.dt.float32
    BF16 = mybir.dt.bfloat16
    FP16 = mybir.dt.float16
    Exp = mybir.ActivationFunctionType.Exp
    Alu = mybir.AluOpType
    seq = env["seq"]
    seq_t, dim_t = env["seq_t"], env["dim_t"]
    n_qc, n_hp, n_m = env["n_qc"], env["n_hp"], env["n_m"]
    xT, QT, ZKA, ZKB = env["xT"], env["QT"], env["ZKA"], env["ZKB"]
    Vaug, UT = env["Vaug"], env["UT"]
    wqr, wkr, wvr, wor = env["wqr"], env["wkr"], env["wvr"], env["wor"]
    xTall, woall = env["xTall"], env["woall"]
    bq_sb, bk_sb, bv_bc = env["bq_sb"], env["bk_sb"], env["bv_bc"]
    xt_d, wq_d, wk_d, wv_d = env["xt_d"], env["wq_d"], env["wk_d"], env["wv_d"]
    wo_d, out_d, rrs_d = env["wo_d"], env["out_d"], env["rrs_d"]

    with (
        tc.tile_pool(name="epool", bufs=8) as epool,
        tc.tile_pool(name="rbpool", bufs=4) as rbpool,
        tc.tile_pool(name="rpool", bufs=4) as rpool,
        tc.tile_pool(name="outstage", bufs=3) as outstage,
    ):
        # ---- input DMAs: few big strided transfers (HWDGE setup is ~625ns
        # per dma_start, so merge aggressively). xT lands in column chunks so
        # the first projections unblock after ~1MB.
        nc.sync.dma_start(
            out=wqall[:].rearrange("p (d c) -> p d c", c=HDL),
            in_=wq_d[:].rearrange("(d p) c -> p d c", p=128))
        xT3 = xTall[:].rearrange("p (d c) -> p d c", c=seq)
        nc.sync.dma_start(
            out=xT3[:, :, 0:512],
            in_=xt_d[:, 0:512].rearrange("(d p) c -> p d c", p=128))
        nc.sync.dma_start(
            out=wkall[:].rearrange("p (d c) -> p d c", c=HDL),
            in_=wk_d[:].rearrange("(d p) c -> p d c", p=128))
        nc.sync.dma_start(
            out=wvall[:].rearrange("p (d c) -> p d c", c=HDL),
            in_=wv_d[:].rearrange("(d p) c -> p d c", p=128))
        for cc in range(1, n_qc):
            cs = slice(cc * 512, (cc + 1) * 512)
            nc.sync.dma_start(
                out=xT3[:, :, cs],
                in_=xt_d[:, cs].rearrange("(d p) c -> p d c", p=128))
        nc.sync.dma_start(
            out=woall[:].rearrange("p (h c) -> p h c", c=DIM),
            in_=wo_d[:].rearrange("(h p) c -> p h c", p=128))

        # ---- phase 1: dense QKV projections (ScalarE idle; PE runs b2b) ----
        with tc.tile_pool(name="qkvp", bufs=1, space="PSUM") as qkvp:
            chain_ctr = [0]

            def bgtile():
                t = qkvp.tile([128, 512], FP32,
                              tag=f"bg{chain_ctr[0] % 8}",
                              name=f"bg{chain_ctr[0] % 8}")
                chain_ctr[0] += 1
                return t

            def emit_q(m, qc):
                qp = bgtile()
                for d in range(dim_t):
                    nc.tensor.matmul(qp[:], wqr[d][:, m*128:(m+1)*128],
                                     xT[d][:, qc*512:(qc+1)*512],
                                     start=(d == 0), stop=(d == dim_t - 1))
                nc.vector.tensor_scalar(QT[m][:, qc*512:(qc+1)*512], qp[:],
                                        bq_sb[:, m:m+1], None, Alu.add)

            def emit_k(m, kc):
                kp = bgtile()
                for d in range(dim_t):
                    nc.tensor.matmul(kp[:], wkr[d][:, m*128:(m+1)*128],
                                     xT[d][:, kc*512:(kc+1)*512],
                                     start=(d == 0), stop=(d == dim_t - 1))
                # (K + bias) * 1/sqrt(HID), split per head into per-kt
                # zero-padded small tiles (big-tile slice stationaries load
                # slower on this HW)
                for j in range(4):
                    t = kc * 4 + j
                    js = slice(j * 128, (j + 1) * 128)
                    nc.vector.tensor_scalar(ZKA[m][t][0:64, :],
                                            kp[0:64, js],
                                            bk_sb[0:64, m:m+1], 0.125,
                                            Alu.add, Alu.mult)
                    nc.vector.tensor_scalar(ZKB[m][t][64:128, :],
                                            kp[64:128, js],
                                            bk_sb[64:128, m:m+1], 0.125,
                                            Alu.add, Alu.mult)

            def emit_v(st):
                vp = bgtile()
                for d in range(dim_t):
                    nc.tensor.matmul(vp[:], xT[d][:, st*128:(st+1)*128],
                                     wvr[d][:],
                                     start=(d == 0), stop=(d == dim_t - 1))
                va3 = Vaug[st][:].rearrange("p (h c) -> p h c", c=HID + 1)
                nc.vector.tensor_tensor(
                    va3[:, :, 0:HID],
                    vp[:].rearrange("p (h c) -> p h c", c=HID),
                    bv_bc[:].rearrange("p (h c) -> p h c", c=HID),
                    Alu.add)

            # PE warm-up during the input DMAs: garbage matmuls keep the PE
            # busy (and the p-state ramping) until the first weights land.
            for i in range(18):
                wp = bgtile()
                nc.tensor.matmul(wp[:, 0:128], ZKA[0][0][64:128, :],
                                 ZKA[0][1][64:128, :],
                                 start=True, stop=True)

            # column chunk 0 lands first: emit its projections, then the rest
            for m in range(n_m):
                emit_q(m, 0)
                emit_k(m, 0)
            for st in range(4):
                emit_v(st)
            for c in range(1, n_qc):
                for m in range(n_m):
                    emit_q(m, c)
                    emit_k(m, c)
                for st in range(4 * c, 4 * c + 4):
                    emit_v(st)

        # ---- phase 2: attention units (no interleaved projection work; s2
        # rotation depth 3 so scores never wait on the exp 2 steps back) ----
        def finish(hp, qc, ua, ub):
            for hi, (ui, rowbase) in enumerate(((ua, 0), (ub, 64))):
                usb = rbpool.tile([HID + 1, 512], FP32, tag="usb", name="usb")
                nc.vector.tensor_copy(usb[:], ui[:])
                rr = rpool.tile([1, 512], FP32, tag="rr", name="rr")
                nc.vector.reciprocal(rr[:], usb[HID:HID+1, :])
                slot = rrs_d[hp, qc, hi, :]
                nc.sync.dma_start(out=slot, in_=rr[0:1, :])
                rb = rbpool.tile([HID, 512], FP32, tag="rb", name="rb")
                nc.sync.dma_start(
                    out=rb[:],
                    in_=bass.AP(tensor=slot.tensor, offset=slot.offset,
                                ap=[[0, HID], [1, 512]]))
                nc.vector.tensor_tensor(
                    UT[hp][qc][rowbase:rowbase+HID, :],
                    usb[0:HID, :], rb[:], Alu.mult)

        with (
            tc.tile_pool(name="sps", bufs=1, space="PSUM") as sps,
            tc.tile_pool(name="ups", bufs=1, space="PSUM") as ups,
        ):
            ktg = [0]   # global kt counter: s2 tag rotation continues
                        # across unit boundaries so the pipeline never drains

            def emit_unit(hp, qc):
                vca = 2 * hp * (HID + 1)
                vcb = (2 * hp + 1) * (HID + 1)
                ua = ups.tile([HID + 1, 512], FP32, tag="ua", name="ua")
                ub = ups.tile([HID + 1, 512], FP32, tag="ub", name="ub")
                qcols = slice(qc * 512, (qc + 1) * 512)
                # attnV lags the score/exp frontier by 2 kt steps so the PE
                # is never queued right behind an exp it would wait on.
                lag = []
                for kt in range(seq_t):
                    s2 = sps.tile([128, 1024], FP32, tag=f"s{ktg[0] % 3}",
                                  name=f"s{ktg[0] % 3}")
                    ktg[0] += 1
                    nc.tensor.matmul(s2[:, 0:512], ZKA[hp][kt][:],
                                     QT[hp][:, qcols], start=True, stop=True)
                    nc.tensor.matmul(s2[:, 512:1024], ZKB[hp][kt][:],
                                     QT[hp][:, qcols], start=True, stop=True)
                    e2 = epool.tile([128, 1024], FP16, tag="e2", name="e2")
                    nc.scalar.activation(e2[:], s2[:], Exp)
                    if len(lag) >= 2:
                        lag.pop(0)()

                    def attn(e2=e2, kt=kt, ua=ua, ub=ub, vca=vca, vcb=vcb):
                        nc.tensor.matmul(ua[:], Vaug[kt][:, vca:vca+HID+1],
                                         e2[:, 0:512],
                                         start=(kt == 0),
                                         stop=(kt == seq_t - 1))
                        nc.tensor.matmul(ub[:], Vaug[kt][:, vcb:vcb+HID+1],
                                         e2[:, 512:1024],
                                         start=(kt == 0),
                                         stop=(kt == seq_t - 1))
                    lag.append(attn)
                for fn in lag:
                    fn()
                finish(hp, qc, ua, ub)

            for u in range(16):
                hp, qc = divmod(u, 4)
                emit_unit(hp, qc)

        # ---- phase 3: dense output projection ----
        with tc.tile_pool(name="ops", bufs=1, space="PSUM") as ops:
            op_ctr = [0]
            for qc in range(n_qc):
                for sti in range(4):
                    st = qc * 4 + sti
                    ot = outstage.tile([128, DIM], FP32, tag="ot", name="ot")
                    for oc in range(2):
                        op = ops.tile([128, 512], FP32,
                                      tag=f"op{op_ctr[0] % 8}",
                                      name=f"op{op_ctr[0] % 8}")
                        op_ctr[0] += 1
                        for hp in range(n_hp):
                            nc.tensor.matmul(
                                op[:], UT[hp][qc][:, sti*128:(sti+1)*128],
                                wor[hp][:, oc*512:(oc+1)*512],
                                start=(hp == 0), stop=(hp == n_hp - 1))
                        nc.vector.tensor_copy(ot[:, oc*512:(oc+1)*512], op[:])
                    (nc.sync if st % 2 == 0 else nc.gpsimd).dma_start(
                        out=out_d[st*128:(st+1)*128, :], in_=ot[:])


def _get_program():
    global _PROG
    if _PROG is None:
        _PROG = _build_program()
    return _PROG


def _make_in_maps(inputs):
    import ml_dtypes
    bf16 = ml_dtypes.bfloat16
    x = np.asarray(inputs["x"], dtype=np.float32)
    xts = [np.ascontiguousarray(x[b].T).astype(bf16) for b in range(B)]
    wq = np.asarray(inputs["Wq"], np.float32).astype(bf16)
    wk = np.asarray(inputs["Wk"], np.float32).astype(bf16)
    wv = np.asarray(inputs["Wv"], np.float32).astype(bf16)
    wo = np.asarray(inputs["Wo"], np.float32).astype(bf16)
    in_maps = []
    for c in range(N_CORES):
        b, g = divmod(c, 2)
        sl = slice(g * HDL, (g + 1) * HDL)
        in_maps.append({
            "xt": xts[b],
            "wq": np.ascontiguousarray(wq[:, sl]),
            "wk": np.ascontiguousarray(wk[:, sl]),
            "wv": np.ascontiguousarray(wv[:, sl]),
            "bq": np.ascontiguousarray(np.asarray(inputs["bq"], np.float32)[sl]),
            "bk": np.ascontiguousarray(np.asarray(inputs["bk"], np.float32)[sl]),
            "bv": np.ascontiguousarray(np.asarray(inputs["bv"], np.float32)[sl]),
            "wo": np.ascontiguousarray(wo[sl, :]),
        })
    return in_maps


def kernel(x, Wq, bq, Wk, bk, Wv, bv, Wo, bo):
    from concourse.bass_utils import run_bass_kernel_spmd

    bo = np.asarray(bo, dtype=np.float32)
    nc = _get_program()
    in_maps = _make_in_maps(dict(x=x, Wq=Wq, bq=bq, Wk=Wk, bk=bk, Wv=Wv,
                                 bv=bv, Wo=Wo, bo=bo))
    res = run_bass_kernel_spmd(nc, in_maps, core_ids=list(range(N_CORES)))
    out = np.empty((B, SEQ, DIM), dtype=np.float32)
    for b in range(B):
        out[b] = res.results[2 * b]["out"] + res.results[2 * b + 1]["out"] + bo
    return out
